# revision 26
# baseline (speedup 1.0000x reference)
"""GCNII layer on 8 TRN2 NeuronCores (Bass/Tile).

Strategy: nodes are assigned to 784 (core, chunk) bins by a greedy 4-vector
bin-packer that balances each bin's per-subrange in-edge counts to <= 512, so
nearly every (chunk, subrange) bucket is exactly 4 gather tiles (the int16
dma_gather index limit forces 4 table subranges of 25000 rows).  The bf16
degree-prenormalized feature table is replicated per core; gathers are merged
into 4 calls per 7-chunk group to amortize SWDGE cost (the Q7 descriptor
worker at ~2.9ns/idx is the kernel's critical resource, so index count is
minimized everywhere: alpha-initial-residual rows enter through a plain
affine DMA, not a gather).  Per 128-edge tile a pure-0/1 one-hot
(iota == slot) is built on DVE (2/3) or via a 2-op Abs/Relu trick on the
Scalar engine (1/3) and TensorE accumulates psum[feat, slot] += buf.T @ oh.
All scalar factors fold away: rsqrt(deg_src) into the table rows,
alpha*init/(0.9*ndst) into a pseudo-row per slot, 0.5*(I+W.T) into one
epilogue matmul, 0.9*rsqrt(deg_dst) into the final ReLU scale (relu commutes
with positive per-row scaling).  Host does integer bucketing/layout and
input preconditioning only.
"""

import sys

if "/opt/trn_rl_repo" not in sys.path:
    sys.path.insert(0, "/opt/trn_rl_repo")

from contextlib import ExitStack

import ml_dtypes
import numpy as np

N, E, D, NC = 100000, 1600000, 128, 8
CHUNKS = 98              # chunks of 128 output slots per core
SLOTS = CHUNKS * 128     # node slots per core: 12544
NBINS = NC * CHUNKS      # 784 (core, chunk) bins
ALPHA = 0.1
NSUB = 4                 # feature-table subranges (int16 index limit)
SR = 25000               # rows per subrange
CAPB = 512               # bucket edge-count target (4 tiles)
SZ = [7] * 12 + [7, 4, 3]   # chunks per gather group (small tail groups)
NGRP = len(SZ)
G0 = [sum(SZ[:i]) for i in range(NGRP)]   # first chunk of each group
SCALAR_EVERY = 2         # every k-th one-hot built on the scalar engine

F32 = np.float32
BF16 = ml_dtypes.bfloat16


def _wrap_idx(seq):
    """dma_gather index layout: i -> [i % 16, i // 16], replicated to 128
    partitions (one copy per Q7 core)."""
    blk = seq.reshape(-1, 16).T
    return np.tile(blk, (8, 1))


def _assign_nodes(dvec):
    """Greedy min-max 4-vector bin packing: nodes (desc by degree) into 784
    bins of 128 slots, keeping every bin's per-subrange sums <= CAPB."""
    tot = dvec.sum(1)
    order = np.argsort(-tot, kind="stable")
    cap = np.full(NBINS, 128, np.int64)
    S = np.zeros((NBINS, NSUB), np.int64)
    assign = np.empty(N, np.int64)
    for i in order:
        d = dvec[i]
        cand = np.flatnonzero(cap > 0)
        Sn = S[cand] + d
        sc = (Sn > CAPB).any(axis=1) * 1e12 + Sn.max(axis=1) * 1e4 + (
            128 - cap[cand])
        j = cand[np.argmin(sc)]
        assign[i] = j
        S[j] += d
        cap[j] -= 1
    return assign, S


def _plan_layout(T):
    """T: [CHUNKS, NSUB] tiles per bucket (shared by all cores).  Buf column
    layout per group g: [r=0: c0..c6][r=1: ...]..[r=3][A: c0..c6]."""
    col = 0
    boff = np.zeros((CHUNKS, NSUB), np.int64)
    aoff = np.zeros(CHUNKS, np.int64)
    calls = []                                  # (col0, ntiles, r, g)
    for g in range(NGRP):
        for r in range(NSUB):
            c0 = col
            for ci in range(SZ[g]):
                c = G0[g] + ci
                boff[c, r] = col
                col += T[c, r]
            calls.append((c0, col - c0, r, g))
        for ci in range(SZ[g]):
            aoff[G0[g] + ci] = col
            col += 1
    return boff, aoff, calls, col


def _host_prep(features, initial_features, W, src, dst):
    src = np.ascontiguousarray(src).astype(np.int64, copy=False)
    dst = np.ascontiguousarray(dst).astype(np.int64, copy=False)
    deg = np.bincount(dst, minlength=N)
    degc = np.maximum(deg, 1).astype(F32)
    norm = 1.0 / np.sqrt(degc)
    table = (features * norm[:, None]).astype(BF16)
    W2 = (0.5 * (np.eye(D, dtype=F32) + W.T)).astype(BF16)

    sub = src // SR
    dvec = np.zeros((N, NSUB), np.int64)
    np.add.at(dvec, (dst, sub), 1)
    assign, S = _assign_nodes(dvec)

    # deal sorted bins to (class=p//8, core=p%8) so similar tile-vectors
    # share a class; plan T = per-class max over cores
    Tb = -(-S // 128)
    keys = Tb[:, 0] * 10 ** 6 + Tb[:, 1] * 10 ** 4 + Tb[:, 2] * 100 + Tb[:, 3]
    bo = np.argsort(keys, kind="stable")
    core_of_bin = np.empty(NBINS, np.int64)
    class_of_bin = np.empty(NBINS, np.int64)
    core_of_bin[bo] = np.arange(NBINS) % NC
    class_of_bin[bo] = np.arange(NBINS) // NC
    T = np.zeros((CHUNKS, NSUB), np.int64)
    for b in range(NBINS):
        np.maximum(T[class_of_bin[b]], Tb[b], out=T[class_of_bin[b]])
    boff, aoff, calls, ntiles = _plan_layout(T)

    # slots: nodes of each bin in id order -> slot 0..127
    bin_nodes_order = np.lexsort((np.arange(N), assign))
    slot_in_bin = np.zeros(N, np.int64)
    binsz = np.bincount(assign, minlength=NBINS)
    starts = np.zeros(NBINS, np.int64)
    np.cumsum(binsz[:-1], out=starts[1:])
    slot_in_bin[bin_nodes_order] = np.arange(N) - starts[assign[
        bin_nodes_order]]
    node_core = core_of_bin[assign]
    node_class = class_of_bin[assign]
    node_slot = node_class * 128 + slot_in_bin       # slot within core

    # per-core glob: slot -> node id (-1 pad)
    glob = np.full((NC, SLOTS), -1, np.int64)
    glob[node_core, node_slot] = np.arange(N)

    e_core = node_core[dst]
    e_chunk = node_class[dst]
    e_slot = slot_in_bin[dst]

    per_core = []
    for c_id in range(NC):
        em = e_core == c_id
        es, ec, el, er = src[em], e_chunk[em], e_slot[em], sub[em]
        grp_of = np.repeat(np.arange(NGRP), SZ)
        okey = grp_of[ec] * (NSUB * CHUNKS) + er * CHUNKS + ec
        o = np.argsort(okey, kind="stable")
        es, ec, el, er, okey = es[o], ec[o], el[o], er[o], okey[o]
        nk = NGRP * NSUB * CHUNKS
        sgeom = np.zeros(nk, np.int64)
        np.cumsum(np.bincount(okey, minlength=nk)[:-1], out=sgeom[1:])
        pos = np.arange(len(es)) - sgeom[okey]
        flatpos = boff[ec, er] * 128 + pos
        idx_flat = np.zeros(ntiles * 128, np.int16)
        rel_flat = np.full(ntiles * 128, -1.0, F32)
        idx_flat[flatpos] = (es - er * SR).astype(np.int16)
        rel_flat[flatpos] = el
        idx_dev = np.concatenate(
            [_wrap_idx(idx_flat[c0 * 128:(c0 + nt) * 128])
             for (c0, nt, _, _) in calls], axis=1).astype(np.int16)
        rel_dev = np.ascontiguousarray(rel_flat.reshape(ntiles, 128).T)

        gl = glob[c_id]
        v = gl >= 0
        a2 = np.zeros((SLOTS, D), F32)
        a2[v] = (ALPHA / 0.9) * initial_features[gl[v]] / norm[gl[v], None]
        scl = np.ones(SLOTS, F32)
        scl[v] = 0.9 * norm[gl[v]]
        per_core.append(dict(
            eidx=np.ascontiguousarray(idx_dev),
            rel=rel_dev,
            nrel=np.ascontiguousarray(-rel_dev),
            a2=a2.astype(BF16),
            scl=np.ascontiguousarray(scl.reshape(CHUNKS, 128).T),
            glob=gl,
        ))
    plan = dict(T=T, boff=boff, aoff=aoff, calls=calls, ntiles=ntiles)
    return per_core, plan, table, W2


_BUILD_CACHE = {}


def _build(plan):
    key = tuple(plan["T"].reshape(-1).tolist())
    if key in _BUILD_CACHE:
        return _BUILD_CACHE[key]
    import concourse.bacc as bacc
    import concourse.bass as bass  # noqa: F401
    import concourse.mybir as mybir
    import concourse.tile as tile

    f32 = mybir.dt.float32
    bf16 = mybir.dt.bfloat16
    i16 = mybir.dt.int16
    Alu = mybir.AluOpType
    Act = mybir.ActivationFunctionType

    T, boff, aoff, calls = plan["T"], plan["boff"], plan["aoff"], plan["calls"]
    ntiles = plan["ntiles"]
    IDXC = sum(nt for (_, nt, _, _) in calls) * 8

    nc = bacc.Bacc("TRN2", target_bir_lowering=False, num_swdge_queues=4)
    feats = nc.dram_tensor("feats", [N, D], bf16, kind="ExternalInput")
    a2d = nc.dram_tensor("a2", [SLOTS, D], bf16, kind="ExternalInput")
    w2d = nc.dram_tensor("w2", [D, D], bf16, kind="ExternalInput")
    iota = nc.dram_tensor("iota", [128, 128], bf16, kind="ExternalInput")
    iotaf = nc.dram_tensor("iotaf", [128, 128], f32, kind="ExternalInput")
    ident = nc.dram_tensor("ident", [128, 128], bf16, kind="ExternalInput")
    eidx = nc.dram_tensor("eidx", [128, IDXC], i16, kind="ExternalInput")
    reld = nc.dram_tensor("rel", [128, ntiles], f32, kind="ExternalInput")
    nreld = nc.dram_tensor("nrel", [128, ntiles], f32, kind="ExternalInput")
    scld = nc.dram_tensor("scl", [128, CHUNKS], f32, kind="ExternalInput")
    out = nc.dram_tensor("out", [SLOTS, D], f32, kind="ExternalOutput")

    a2v = a2d.rearrange("(c p) d -> p c d", p=128)     # slot-major -> affine

    gc_max = 0
    for g in range(NGRP):
        g0 = calls[g * NSUB][0]
        g1 = aoff[G0[g] + SZ[g] - 1] + 1
        gc_max = max(gc_max, int(g1 - g0))

    with tile.TileContext(nc) as tc, ExitStack() as ctx:
        const = ctx.enter_context(tc.tile_pool(name="const", bufs=1))
        bufp = ctx.enter_context(tc.tile_pool(name="buf", bufs=3))
        ohp = ctx.enter_context(tc.tile_pool(name="oh", bufs=112))
        abspool = ctx.enter_context(tc.tile_pool(name="abs", bufs=32))
        hp = ctx.enter_context(tc.tile_pool(name="hY", bufs=4))
        op = ctx.enter_context(tc.tile_pool(name="ob", bufs=4))
        ps1 = ctx.enter_context(tc.tile_pool(name="ps1", bufs=4, space="PSUM"))
        ps2 = ctx.enter_context(tc.tile_pool(name="ps2", bufs=4, space="PSUM"))

        # per-group index tiles (distinct tags -> distinct slots) so the
        # first gather only waits for its own group's index DMA
        idx_g = []
        for g in range(NGRP):
            base = sum(x[1] for x in calls[:g * NSUB]) * 8
            gcols = sum(calls[g * NSUB + k][1] for k in range(NSUB)) * 8
            t_ = const.tile([128, gcols], i16, tag=f"ix{g}", name=f"ix{g}")
            nc.sync.dma_start(out=t_[:], in_=eidx[:, base:base + gcols])
            idx_g.append((t_, base))
        iota_sb = const.tile([128, 128], bf16)
        nc.sync.dma_start(out=iota_sb[:], in_=iota[:])
        iota_f = const.tile([128, 128], f32)
        nc.sync.dma_start(out=iota_f[:], in_=iotaf[:])
        id_sb = const.tile([128, 128], bf16)
        nc.sync.dma_start(out=id_sb[:], in_=ident[:])
        w2_sb = const.tile([128, 128], bf16)
        nc.sync.dma_start(out=w2_sb[:], in_=w2d[:])
        rel_sb = const.tile([128, ntiles], f32)
        nc.sync.dma_start(out=rel_sb[:], in_=reld[:])
        nrel_sb = const.tile([128, ntiles], f32)
        nc.sync.dma_start(out=nrel_sb[:], in_=nreld[:])
        scl_sb = const.tile([128, CHUNKS], f32)
        nc.sync.dma_start(out=scl_sb[:], in_=scld[:])

        oh_i = 0
        for g in range(NGRP):
            g0 = calls[g * NSUB][0]
            buf = bufp.tile([128, gc_max * 128], bf16)
            for k in range(NSUB):
                c0, nt, r, _ = calls[g * NSUB + k]
                if nt == 0:
                    continue
                ni = nt * 128
                off = c0 - g0
                gt, gbase = idx_g[g]
                cb = sum(x[1] for x in calls[:g * NSUB + k]) * 8 - gbase
                lo = r * SR
                nc.gpsimd.dma_gather(
                    out_ap=buf[:, off * 128:(off + nt) * 128]
                    .rearrange("p (t d) -> p t d", t=nt),
                    in_ap=feats[lo:lo + SR, :],
                    idxs_ap=gt[:, cb:cb + nt * 8],
                    num_idxs=ni,
                    num_idxs_reg=ni,
                    elem_size=D,
                    single_packet=False,
                    queue_num=(g * NSUB + k) % 4,
                )
            # alpha-init pseudo rows: plain affine DMA, no gather
            gs = SZ[g]
            a_off = int(aoff[G0[g]]) - g0
            nc.sync.dma_start(
                out=buf[:, a_off * 128:(a_off + gs) * 128]
                .rearrange("p (c d) -> p c d", c=gs),
                in_=a2v[:, G0[g]:G0[g] + gs, :],
            )
            for ci in range(gs):
                c = G0[g] + ci
                psum = ps1.tile([128, 128], f32, space="PSUM")
                k = 0
                for r in range(NSUB):
                    for t in range(int(T[c, r])):
                        j = int(boff[c, r]) + t
                        oh = ohp.tile([128, 128], bf16)
                        if (g < NGRP - 1
                                and oh_i % SCALAR_EVERY == SCALAR_EVERY - 1):
                            ab = abspool.tile([128, 128], f32)
                            nc.scalar.activation(
                                ab[:], iota_f[:], Act.Abs,
                                bias=nrel_sb[:, j:j + 1])
                            nc.scalar.activation(
                                oh[:], ab[:], Act.Relu, bias=1.0, scale=-1.0)
                        else:
                            nc.vector.tensor_scalar(
                                oh[:], iota_sb[:], rel_sb[:, j:j + 1], None,
                                Alu.is_equal)
                        oh_i += 1
                        jo = j - g0
                        nc.tensor.matmul(
                            psum[:],
                            lhsT=buf[:, jo * 128:(jo + 1) * 128],
                            rhs=oh[:],
                            start=(k == 0),
                            stop=False,
                        )
                        k += 1
                ja = int(aoff[c]) - g0
                nc.tensor.matmul(
                    psum[:],
                    lhsT=buf[:, ja * 128:(ja + 1) * 128],
                    rhs=id_sb[:],
                    start=(k == 0),
                    stop=True,
                )
                hY = hp.tile([128, 128], bf16)
                nc.scalar.activation(hY[:], psum[:], Act.Copy)
                psO = ps2.tile([128, 128], f32, space="PSUM")
                nc.tensor.matmul(psO[:], lhsT=hY[:], rhs=w2_sb[:],
                                 start=True, stop=True)
                ob = op.tile([128, 128], f32)
                nc.scalar.activation(ob[:], psO[:], Act.Relu,
                                     scale=scl_sb[:, c:c + 1])
                nc.sync.dma_start(out=out[c * 128:(c + 1) * 128, :],
                                  in_=ob[:])

    nc.compile()
    _BUILD_CACHE[key] = nc
    return nc


def _install_ntff_shim():
    """antenv.axon_hooks is absent in this image; shim it and wire the real
    NTFF profiling hook via ctypes so trace=True works under axon."""
    import contextlib
    import ctypes
    import types

    try:
        from antenv import axon_hooks  # noqa: F401
        return
    except ImportError:
        pass
    import antenv

    mod = types.ModuleType("antenv.axon_hooks")
    _hook = [None]
    mod.set_axon_ntff_profile_hook = lambda h: _hook.__setitem__(0, h)
    mod.get_axon_ntff_profile_hook = lambda: _hook[0]
    sys.modules["antenv.axon_hooks"] = mod
    antenv.axon_hooks = mod
    try:
        lib = ctypes.CDLL("/opt/axon/libaxon_pjrt.so")
    except OSError:
        return
    if not hasattr(lib, "axon_start_nrt_profile"):
        return
    lib.axon_start_nrt_profile.argtypes = [
        ctypes.POINTER(ctypes.c_int64),
        ctypes.c_size_t,
    ]
    lib.axon_start_nrt_profile.restype = ctypes.c_int64
    lib.axon_stop_nrt_profile.argtypes = [ctypes.c_char_p]
    lib.axon_stop_nrt_profile.restype = ctypes.c_int64

    @contextlib.contextmanager
    def _hook_cm(output_dir, device_ids):
        import jax

        jax.devices()
        if device_ids:
            ids = (ctypes.c_int64 * len(device_ids))(*device_ids)
            rc = lib.axon_start_nrt_profile(ids, len(device_ids))
        else:
            rc = lib.axon_start_nrt_profile(None, 0)
        if rc != 0:
            raise RuntimeError(f"axon_start_nrt_profile rc={rc}")
        try:
            yield
        finally:
            rc = lib.axon_stop_nrt_profile(output_dir.encode())
            if rc != 0:
                print(f"WARNING: axon_stop_nrt_profile rc={rc}", flush=True)

    mod.set_axon_ntff_profile_hook(_hook_cm)


def _run(inputs, trace=False, trace_cores=None):
    from concourse import bass_utils

    if trace:
        _install_ntff_shim()
    features = np.ascontiguousarray(np.asarray(inputs["features"], dtype=F32))
    initial_features = np.ascontiguousarray(
        np.asarray(inputs["initial_features"], dtype=F32)
    )
    W = np.asarray(inputs["W"], dtype=F32)
    src = np.asarray(inputs["src"])
    dst = np.asarray(inputs["dst"])
    per_core, plan, table, W2 = _host_prep(
        features, initial_features, W, src, dst)
    nc = _build(plan)
    iota_f32 = np.ascontiguousarray(
        np.tile(np.arange(128, dtype=F32), (128, 1)))
    iota_np = np.ascontiguousarray(iota_f32.astype(BF16))
    ident_np = np.eye(128, dtype=F32).astype(BF16)
    in_maps = []
    for c in range(NC):
        pc = per_core[c]
        in_maps.append(dict(
            feats=table,
            a2=pc["a2"],
            w2=W2,
            iota=iota_np,
            iotaf=iota_f32,
            ident=ident_np,
            eidx=pc["eidx"],
            rel=pc["rel"],
            nrel=pc["nrel"],
            scl=pc["scl"],
        ))
    res = bass_utils.run_bass_kernel_spmd(
        nc,
        in_maps,
        core_ids=list(range(NC)),
        trace=trace,
        trace_cores=trace_cores,
    )
    result = np.empty((N, D), F32)
    for c in range(NC):
        gl = per_core[c]["glob"]
        oc = res.results[c]["out"]
        v = gl >= 0
        result[gl[v]] = oc[v]
    return result, res


def kernel(**inputs):
    return _run(inputs, trace=False)[0]
